# revision 46
# baseline (speedup 1.0000x reference)
"""CPSF Memcell Autoencoder on 8 Trainium2 cores — pure data parallel.

Per-core: 1 image [3,256,256]. Encoder (2 conv paths) -> memcell
(softmax retrieval over 32 slots, global delta-rule V update via
AllGather of per-core dV) -> deconv decoder back to [3,256,256].
"""
import sys
sys.path.insert(0, '/opt/trn_rl_repo')
import numpy as np
import concourse.bass as bass
import concourse.bacc as bacc
import concourse.mybir as mybir
import concourse.tile as tile
from concourse import masks
from concourse.bass_utils import run_bass_kernel_spmd

f32 = mybir.dt.float32
f32r = mybir.dt.float32r
f16 = mybir.dt.float16
AF = mybir.ActivationFunctionType
ALU = mybir.AluOpType

N_CORES = 8
N, M, S = 16, 32, 128
ALPHA = 1e-06
H = W = 256          # full res
NSTRIP = 8           # encoder strips
SR = 32              # conv1 out rows per strip (plus 2 halo rows)
RW = 258             # padded row width in a1 buffers
R1 = 34              # conv1 rows per strip incl halo

_cache = {}


def _mk_ap(tile_ap, offset, dims):
    """Manual AP: partition dim from tile_ap, then free dims [[step,count],...]."""
    part = list(tile_ap.ap[0])
    return bass.AP(tile_ap.tensor, offset, [part] + [list(d) for d in dims])


def _build():
    nc = bacc.Bacc("TRN2", target_bir_lowering=False)

    # ---------------- DRAM I/O ----------------
    x_d = nc.dram_tensor("x", [3, H, W], f32r, kind="ExternalInput")
    w1s_d = nc.dram_tensor("w1s", [27, 128], f32r, kind="ExternalInput")
    w1n_d = nc.dram_tensor("w1n", [27, 16], f32r, kind="ExternalInput")
    w2s_d = nc.dram_tensor("w2s", [128, 9 * 128], f16, kind="ExternalInput")
    w2n3_d = nc.dram_tensor("w2n3", [48, 48], f16, kind="ExternalInput")
    ckt_d = nc.dram_tensor("ckt", [16, 32], f16, kind="ExternalInput")
    v_d = nc.dram_tensor("vmat", [32, 128], f32r, kind="ExternalInput")
    decw_d = nc.dram_tensor("decw", [128, 9 * 12], f16, kind="ExternalInput")
    w3s_d = nc.dram_tensor("w3s", [48, 12], f16, kind="ExternalInput")
    b1s_d = nc.dram_tensor("b1s", [128, 1], f32, kind="ExternalInput")
    b1n_d = nc.dram_tensor("b1n", [16, 1], f32, kind="ExternalInput")
    b2s_d = nc.dram_tensor("b2s", [128, 1], f32, kind="ExternalInput")
    b2n_d = nc.dram_tensor("b2n", [16, 1], f32, kind="ExternalInput")
    bdec_d = nc.dram_tensor("bdec", [12, 1], f32, kind="ExternalInput")
    b3r_d = nc.dram_tensor("b3r", [12, 1], f32, kind="ExternalInput")
    out_d = nc.dram_tensor("out", [3, H, W], f32, kind="ExternalOutput")

    with tile.TileContext(nc) as tc:
        with (
            tc.tile_pool(name="pconst", bufs=1) as pc,
            tc.tile_pool(name="ppersist", bufs=1) as pp,
            tc.tile_pool(name="pdram", bufs=1, space="DRAM") as pdram,
        ):
            # ------- constants (conv1 weights first so PE can start ASAP;
            # later-phase consts go on the scalar queue) -------
            w1s = pc.tile([27, 128], f32r); nc.sync.dma_start(w1s[:], w1s_d[:])
            w1n = pc.tile([27, 16], f32r); nc.sync.dma_start(w1n[:], w1n_d[:])
            b1s = pc.tile([128, 1], f32); nc.sync.dma_start(b1s[:], b1s_d[:])
            b1n = pc.tile([16, 1], f32); nc.sync.dma_start(b1n[:], b1n_d[:])
            w2s = pc.tile([128, 9 * 128], f16)
            nc.scalar.dma_start(w2s[:], w2s_d[:])
            w2n3 = pc.tile([48, 48], f16); nc.scalar.dma_start(w2n3[:], w2n3_d[:])
            b2s = pc.tile([128, 1], f32); nc.scalar.dma_start(b2s[:], b2s_d[:])
            b2n = pc.tile([16, 1], f32); nc.scalar.dma_start(b2n[:], b2n_d[:])
            ckt = pc.tile([16, 32], f16); nc.scalar.dma_start(ckt[:], ckt_d[:])
            vmat = pc.tile([32, 128], f32r); nc.scalar.dma_start(vmat[:], v_d[:])
            decw = pc.tile([128, 9 * 12], f16)
            nc.scalar.dma_start(decw[:], decw_d[:])
            w3s = pc.tile([48, 12], f16); nc.scalar.dma_start(w3s[:], w3s_d[:])
            bdec = pc.tile([12, 1], f32); nc.scalar.dma_start(bdec[:], bdec_d[:])
            b3r = pc.tile([12, 1], f32); nc.scalar.dma_start(b3r[:], b3r_d[:])
            ident = pc.tile([128, 128], f32)
            masks.make_identity(nc, ident[:])
            identh = pc.tile([128, 128], f16)
            nc.vector.tensor_copy(identh[:], ident[:])

            # ------- persistent across phases -------
            w_h = pp.tile([128, 32 * 128], f16)         # token-major softmax weights
            vnew = pp.tile([32, 128], f16)

            # =====================  ENCODER  =====================
            with (
                tc.tile_pool(name="pE", bufs=1) as pE,
                tc.tile_pool(name="pEd", bufs=1) as pEd,
                tc.tile_pool(name="pst", bufs=2) as pst,
                tc.tile_pool(name="psA", bufs=1, space="PSUM") as psA,
                tc.tile_pool(name="psB", bufs=1, space="PSUM") as psB,
                tc.tile_pool(name="psC2", bufs=2, space="PSUM") as psC2,
            ):
                ps_ag = psB.tile([32, 160], f32, tag="psag")  # [A | G] accumulator

                NR = 32          # full-res rows per strip
                NCH = R1 * 256 // 512          # conv1 px chunks (2 rows each) = 17
                NQ = (NR // 2) * 128 // 512    # conv2 px chunks of 512 = 4

                def emit_im1(s):
                    """im2col loads for conv1 strip s (prefetched a strip
                    ahead: slot (s%2) was last read by conv1(s-2))."""
                    y0 = NR * s
                    im1 = pEd.tile([27, R1 * 256], f32r, tag="im1", bufs=2,
                                   name=f"im1s{s}")
                    # Edge zeroing memset full partition width; tap DMAs then
                    # overwrite everything they own, leaving zeros only where
                    # no tap writes (kx=0 col 0, kx=2 col 255, halo rows).
                    if s < 2:   # one-time per rotating buffer
                        nc.vector.memset(
                            im1[:].bitcast(f32).rearrange(
                                "p (r c) -> p r c", c=256)[:, :, 0:1], 0.0)
                        nc.vector.memset(
                            im1[:].bitcast(f32).rearrange(
                                "p (r c) -> p r c", c=256)[:, :, 255:256], 0.0)
                    if s == 0:          # top halo rows outside the image
                        nc.vector.memset(im1[:, 0:2 * 256].bitcast(f32), 0.0)
                    if s == NSTRIP - 1:  # bottom halo rows
                        nc.vector.memset(
                            im1[:, (R1 - 2) * 256:R1 * 256].bitcast(f32), 0.0)
                    for half in range(2):
                        for ky in range(3):
                            for kx in range(3):
                                t = ky * 3 + kx
                                r_lo = max(0, 2 - y0 - ky, 16 * half)
                                r_hi = min(R1, 258 - y0 - ky,
                                           16 if half == 0 else R1)
                                if r_hi <= r_lo:
                                    continue
                                c_lo = max(0, 1 - kx)
                                c_hi = min(256, 257 - kx)
                                src = x_d[0:3,
                                          y0 - 2 + r_lo + ky: y0 - 2 + r_hi + ky,
                                          c_lo + kx - 1: c_hi + kx - 1]
                                nc.sync.dma_start(
                                    im1[3 * t:3 * t + 3, :]
                                    .rearrange("p (r c) -> p r c", r=R1)
                                    [:, r_lo:r_hi, c_lo:c_hi],
                                    src)
                    return im1

                def emit_ag(s, tstT):
                    # A|G accumulation for strip s's 16 chunks
                    for j in range(16):
                        c = 16 * s + j
                        lhs = w_h[:, 32 * c:32 * c + 32]
                        nc.tensor.matmul(ps_ag[:, 0:128], lhs,
                                         tstT[:, 128 * j:128 * (j + 1)],
                                         start=(c == 0), stop=(c == 127))
                        nc.tensor.matmul(ps_ag[:, 128:160], lhs, lhs,
                                         start=(c == 0), stop=(c == 127))

                pending_ag = None
                im1_next = emit_im1(0)
                for s in range(NSTRIP):
                    y0 = NR * s
                    im1 = im1_next
                    if s + 1 < NSTRIP:
                        im1_next = emit_im1(s + 1)

                    # a1s: [128, 34 rows x 258 cols] fp16, col 0/257 zero pads
                    a1s = pEd.tile([128, R1 * RW], f16, tag="a1s", bufs=2)
                    # a1n3: [48, 34 x 258]: group g rows 16g..16g+16 hold a1n
                    # row (r+g) at row-block r (built by 2 shift-copy DMAs)
                    a1n3 = pEd.tile([48, R1 * RW], f16, tag="a1n3", bufs=2)
                    if s < 2:   # one-time pad zeroing of the rotating buffers
                        nc.vector.memset(
                            a1s[:].rearrange("p (r c) -> p r c", c=RW)[:, :, 0:1], 0.0)
                        nc.vector.memset(
                            a1s[:].rearrange("p (r c) -> p r c", c=RW)[:, :, 257:258], 0.0)
                        nc.vector.memset(
                            a1n3[0:16, :].rearrange("p (r c) -> p r c", c=RW)[:, :, 0:1], 0.0)

                    # ---- conv1 wide + narrow ----
                    for i2 in range(9):          # 8 pairs + 1 single chunk
                        lo = 1024 * i2
                        npx = 1024 if i2 < 8 else 512
                        c1 = psB.tile([128, 1024], f32, tag="c1")
                        nc.tensor.matmul(c1[:, 0:512], w1s[:], im1[:, lo:lo + 512],
                                         start=True, stop=True)
                        if npx == 1024:
                            nc.tensor.matmul(c1[:, 512:1024], w1s[:],
                                             im1[:, lo + 512:lo + 1024],
                                             start=True, stop=True)
                        nrow = npx // 256
                        nc.scalar.activation(
                            a1s[:, :].rearrange("p (r c) -> p r c", c=RW)
                            [:, 4 * i2:4 * i2 + nrow, 1:257],
                            c1[:, 0:npx].rearrange("p (r c) -> p r c", r=nrow),
                            AF.Silu, bias=b1s[:])
                        for h in range(npx // 512):
                            cn = psB.tile([16, 512], f32, tag="cn")
                            nc.tensor.matmul(cn[:], w1n[:],
                                             im1[:, lo + 512 * h:lo + 512 * (h + 1)],
                                             start=True, stop=True)
                            nc.scalar.activation(
                                a1n3[0:16, :].rearrange("p (r c) -> p r c", c=RW)
                                [:, 2 * (2 * i2 + h):2 * (2 * i2 + h) + 2, 1:257],
                                cn[:].rearrange("p (r c) -> p r c", r=2),
                                AF.Silu, bias=b1n[:])
                        if i2 == 4:
                            # rows [0,17) done: first-half replicas (these
                            # read rows >= g >= 1, so no row-0 pad conflict)
                            for g in (1, 2):
                                nc.gpsimd.dma_start(
                                    a1n3[16 * g:16 * (g + 1), 0:(17 - g) * RW],
                                    a1n3[0:16, g * RW:17 * RW])
                    if s == 0:      # conv2 zero-pad at image top: a1 row 0
                        nc.vector.memset(a1s[:, 1:257], 0.0)
                        nc.vector.memset(a1n3[0:16, 1:257], 0.0)
                    if s == NSTRIP - 1:  # bottom: row R1-1
                        nc.vector.memset(
                            a1s[:, (R1 - 1) * RW + 1:(R1 - 1) * RW + 257], 0.0)
                        nc.vector.memset(
                            a1n3[0:16, (R1 - 1) * RW + 1:(R1 - 1) * RW + 257], 0.0)
                    # second-half replicas: a1n rows [17, 34)
                    for g in (1, 2):
                        nc.gpsimd.dma_start(
                            a1n3[16 * g:16 * (g + 1), (17 - g) * RW:(R1 - g) * RW],
                            a1n3[0:16, 17 * RW:R1 * RW])

                    # ---- conv2 narrow -> z  (3 accumulating matmuls per chunk) ----
                    NPX2 = (NR // 2) * 128      # conv2 out px per strip
                    z_fl = pE.tile([16, NPX2], f16, tag="z")
                    for q in range(NQ):
                        cn = psB.tile([16, 512], f32, tag="cn")
                        for kxi, kx in enumerate((0, 1, 2)):
                            off = RW * 8 * q + kx
                            rhs = _mk_ap(a1n3[:], off, [[2 * RW, 4], [2, 128]])
                            nc.tensor.matmul(cn[:], w2n3[:, 16 * kx:16 * (kx + 1)],
                                             rhs, start=(kxi == 0), stop=(kxi == 2))
                        nc.scalar.activation(z_fl[:, 512 * q:512 * (q + 1)], cn[:],
                                             AF.Silu, bias=b2n[:])

                    if pending_ag is not None:   # A|G of strip s-1
                        emit_ag(*pending_ag)
                        pending_ag = None

                    # ---- conv2 wide + silu + transpose + logits ----
                    tstT = pE.tile([128, 2048], f16, tag="tstT", bufs=2)
                    ps_log = psA.tile([128, 32 * 4 * NQ], f32, tag="pslog")
                    for q in range(NQ):
                        c2 = psC2.tile([128, 512], f32, tag="c2s")
                        for t9 in range(9):
                            ky, kx = t9 // 3, t9 % 3
                            rhs = a1s[:, :].rearrange("p (r c) -> p r c", c=RW)[
                                :, 8 * q + ky: 8 * q + ky + 8: 2, kx: kx + 256: 2]
                            nc.tensor.matmul(c2[:], w2s[:, 128 * t9:128 * (t9 + 1)],
                                             rhs, start=(t9 == 0), stop=(t9 == 8))
                        ts_t = pst.tile([128, 512], f16, tag="tst")
                        nc.scalar.activation(ts_t[:], c2[:], AF.Silu, bias=b2s[:])
                        ps_tr = psB.tile([128, 512], f16, tag="pstr")
                        for j in range(4):
                            nc.tensor.transpose(ps_tr[:, 128 * j:128 * (j + 1)],
                                                ts_t[:, 128 * j:128 * (j + 1)], identh[:])
                        nc.vector.tensor_copy(
                            tstT[:, 512 * q:512 * (q + 1)], ps_tr[:])
                        for j in range(4):
                            nc.tensor.matmul(
                                ps_log[:, 32 * (4 * q + j):32 * (4 * q + j) + 32],
                                z_fl[0:16, 512 * q + 128 * j: 512 * q + 128 * (j + 1)],
                                ckt[:], start=True, stop=True)
                    # ---- softmax over 32 slots (free dim), strip-batched ----
                    sl = slice(512 * s, 512 * (s + 1))
                    e_st = pst.tile([128, 512], f32, tag="est", bufs=1)
                    nc.scalar.activation(e_st[:], ps_log[:], AF.Exp)
                    den = pst.tile([128, 16], f32, tag="den")
                    nc.vector.tensor_reduce(
                        den[:], e_st[:].rearrange("p (c k) -> p c k", k=32),
                        mybir.AxisListType.X, ALU.add)
                    rec = pst.tile([128, 16], f32, tag="rec")
                    nc.vector.reciprocal(rec[:], den[:])
                    wslice = w_h[:, sl]
                    nc.vector.tensor_tensor(
                        wslice.rearrange("p (c k) -> p c k", k=32),
                        e_st[:].rearrange("p (c k) -> p c k", k=32),
                        rec[:].rearrange("p (c k) -> p c k", k=1).broadcast_to([128, 16, 32]),
                        ALU.mult)

                    # A|G for this strip is deferred into the next strip's
                    # emission (hides the softmax chain under conv1 PE work)
                    pending_ag = (s, tstT)
                if pending_ag is not None:
                    emit_ag(*pending_ag)

                # ---- dV + collective (still inside encoder pools) ----
                a_sb = pst.tile([32, 128], f32, tag="asb", bufs=1)
                nc.vector.tensor_copy(a_sb[:], ps_ag[:, 0:128])
                g_sb = pst.tile([32, 32], f32r, tag="gsb", bufs=1)
                nc.vector.tensor_copy(g_sb[:], ps_ag[:, 128:160])
                ps_gv_t = psC2.tile([128, 512], f32, tag="c2s", name="psgv")
                ps_gv = ps_gv_t[0:32, 0:128]
                nc.tensor.matmul(ps_gv[:], g_sb[:], vmat[:], start=True, stop=True)
                dv_sb = pst.tile([32, 128], f16, tag="dvsb", bufs=1)
                nc.vector.tensor_sub(dv_sb[:], a_sb[:], ps_gv[:])
                dv_in = pdram.tile([32, 128], f16)
                dv_out = pdram.tile([32 * N_CORES, 128], f16)
                nc.sync.dma_start(dv_in[:], dv_sb[:])
                nc.gpsimd.collective_compute(
                    "AllGather", ALU.bypass,
                    replica_groups=[list(range(N_CORES))],
                    ins=[dv_in.opt()], outs=[dv_out.opt()])

            # =====================  DECODER  =====================
            with (
                tc.tile_pool(name="pD", bufs=1) as pD,
                tc.tile_pool(name="pst2", bufs=2) as pst2,
                tc.tile_pool(name="psC", bufs=2, space="PSUM") as psC,
                tc.tile_pool(name="psD", bufs=2, space="PSUM") as psD,
            ):
                # ---- w slot-major via PE transpose (overlaps the AllGather) ----
                w_sT = pD.tile([32, 16384], f16)
                for g in range(32):           # 4 transposes per psum tile
                    ps_wt = psC.tile([128, 512], f16, tag="psCw", name="ps_wt")[0:32, :]
                    for j in range(4):
                        c = 4 * g + j
                        nc.tensor.transpose(ps_wt[:, 128 * j:128 * (j + 1)],
                                            w_h[:, 32 * c:32 * c + 32], identh[:])
                    nc.vector.tensor_copy(w_sT[:, 512 * g:512 * (g + 1)], ps_wt[:])

                # d0m: padded [130,130] fp16; borders zeroed (overlaps AllGather)
                d0m = pD.tile([128, 130 * 130], f16)
                nc.gpsimd.memset(d0m[:, 0:130], 0.0)
                nc.gpsimd.memset(d0m[:, 129 * 130:130 * 130], 0.0)
                nc.gpsimd.memset(
                    d0m[:].rearrange("p (r c) -> p r c", c=130)[:, :, 0:1], 0.0)
                nc.gpsimd.memset(
                    d0m[:].rearrange("p (r c) -> p r c", c=130)[:, :, 129:130], 0.0)
                # img12: deconv output, parity planes [12, 130, 130] fp16 in SBUF
                # (1-element front pad + tail slack: im3d copies read a full
                # 128x130-element span at shift offsets in [-1, 261])
                img12p = pD.tile([12, 130 * 130 + 6], f16)
                img12 = img12p[:, 1:1 + 130 * 130]
                nc.gpsimd.memset(img12[:, 0:130], 0.0)
                nc.gpsimd.memset(img12[:, 129 * 130:130 * 130], 0.0)
                nc.gpsimd.memset(
                    img12.rearrange("p (r c) -> p r c", c=130)[:, :, 0:1], 0.0)
                nc.gpsimd.memset(
                    img12.rearrange("p (r c) -> p r c", c=130)[:, :, 129:130], 0.0)

                # ---- gather dV, reduce, vnew (after AllGather) ----
                gath = pst2.tile([32, 8 * 128], f16, tag="gath", bufs=1)
                nc.sync.dma_start(
                    gath[:].rearrange("p (r c) -> p r c", r=N_CORES),
                    dv_out[:].rearrange("(r p) c -> p r c", p=32))
                nc.vector.tensor_add(gath[:, 0:512], gath[:, 0:512], gath[:, 512:1024])
                nc.vector.tensor_add(gath[:, 0:256], gath[:, 0:256], gath[:, 256:512])
                nc.vector.tensor_add(gath[:, 0:128], gath[:, 0:128], gath[:, 128:256])
                nc.vector.scalar_tensor_tensor(
                    vnew[:], gath[:, 0:128], ALPHA, vmat[:],
                    op0=ALU.mult, op1=ALU.add)

                # ---- t_read + deconv interleaved ----
                # t_read chunk q fills d0m rows 4q+1..4q+5; deconv chunk q needs
                # rows 4q..4q+6 -> run deconv one chunk behind.
                def t_read(q):
                    ps_rd = psC.tile([128, 512], f32, tag="psC")
                    nc.tensor.matmul(ps_rd[:], vnew[:], w_sT[:, 512 * q:512 * (q + 1)],
                                     start=True, stop=True)
                    nc.vector.tensor_copy(
                        d0m[:].rearrange("p (r c) -> p r c", c=130)
                        [:, 4 * q + 1:4 * q + 5, 1:129],
                        ps_rd[:].rearrange("p (r c) -> p r c", r=4))

                def deconv(q):
                    ps_dec = psD.tile([12, 512], f32, tag="psdec")
                    for t9 in range(9):
                        dy, dx = t9 // 3 - 1, t9 % 3 - 1
                        rhs = d0m[:].rearrange("p (r c) -> p r c", c=130)[
                            :, 4 * q + 1 + dy:4 * q + 5 + dy, 1 + dx:129 + dx]
                        nc.tensor.matmul(ps_dec[:], decw[:, 12 * t9:12 * (t9 + 1)],
                                         rhs, start=(t9 == 0), stop=(t9 == 8))
                    stg = pst2.tile([12, 512], f16, tag="stdec")
                    nc.scalar.activation(stg[:], ps_dec[:], AF.Silu, bias=bdec[:])
                    nc.sync.dma_start(
                        img12.rearrange("p (r c) -> p r c", c=130)
                        [:, 1 + 4 * q:5 + 4 * q, 1:129],
                        stg[:].rearrange("p (r c) -> p r c", r=4))

                # ---- im3d: 16 shifted copies of img12 planes, emitted in two
                # column halves interleaved with deconv so they overlap.
                # row ((a,dy),(b,dx),ch); AY/BX combo order matches host w3s prep
                AY = [(0, 0), (0, 1), (1, -1), (1, 0)]
                im3d = pD.tile([48, 128 * 130], f16)
                QC = 32 * 130    # quarter of the im3d columns

                def im3d_quarter(k):
                    engs = [nc.sync, nc.gpsimd]
                    for iy, (a, dy) in enumerate(AY):
                        for jx, (b, dx) in enumerate(AY):
                            p0 = (a * 2 + b) * 3
                            r0 = 3 * (4 * iy + jx)
                            off = 1 + (1 + dy) * 130 + dx + QC * k
                            engs[(4 * iy + jx) % 2].dma_start(
                                im3d[r0:r0 + 3, QC * k:QC * (k + 1)],
                                img12p[p0:p0 + 3, off:off + QC])

                t_read(0)
                for q in range(32):
                    if q + 1 < 32:
                        t_read(q + 1)
                    deconv(q)
                    if q in (9, 17, 25):
                        im3d_quarter((q - 9) // 8)
                im3d_quarter(3)

                # ---- conv3: two stacked matmuls (one per x-parity b') per
                # 512-px chunk; strided acts interleave x into stg3i ----
                for G in range(8):            # groups of 4 chunks (16 out rows)
                    stg3i = pst2.tile([6, 16 * 256], f32, tag="stg3i")
                    for j in range(4):
                        c = 4 * G + j
                        rhs = _mk_ap(im3d[:], 520 * c + 1, [[130, 4], [1, 128]])
                        for b in range(2):
                            pc3 = psD.tile([6, 512], f32, tag="pc3")
                            nc.tensor.matmul(pc3[:], w3s[:, 6 * b:6 * b + 6],
                                             rhs, start=True, stop=True)
                            nc.scalar.activation(
                                _mk_ap(stg3i[:], 1024 * j + b, [[256, 4], [2, 128]]),
                                pc3[:].rearrange("p (r c) -> p r c", r=4),
                                AF.Silu, bias=b3r[0:6])
                    for a in range(2):        # y-parity class -> row-strided out
                        src = stg3i[3 * a:3 * a + 3, :].rearrange(
                            "p (r c) -> p r c", c=256)
                        dst = out_d[0:3, 32 * G + a:32 * G + a + 31:2, :]
                        nc.sync.dma_start(dst, src)

    nc.compile()
    return nc


def _prep_weights(i):
    """Host-side weight layout prep. i = dict of full inputs."""
    f = np.float32
    h = np.float16
    w1s = np.ascontiguousarray(
        i['e0s_w1'].transpose(2, 3, 1, 0).reshape(27, 128)).astype(f)
    w1n = np.ascontiguousarray(
        i['e0n_w1'].transpose(2, 3, 1, 0).reshape(27, 16)).astype(f)
    w2s = np.ascontiguousarray(
        i['e0s_w2'].transpose(1, 2, 3, 0).reshape(128, 9 * 128)).astype(h)
    # conv2 narrow: per-kx blocks [48, 16]: w2n3[16*ky+ci, 16*kx+co]
    w2n = i['e0n_w2']  # [co, ci, ky, kx]
    w2n3 = np.zeros((48, 48), h)
    for kx in range(3):
        for ky in range(3):
            w2n3[16 * ky:16 * ky + 16, 16 * kx:16 * kx + 16] = \
                w2n[:, :, ky, kx].T.astype(h)
    ckt = (i['cell_k'].T * np.float32(0.25)).astype(h).copy()   # [16,32], /sqrt(16)
    vmat = i['cell_v'].astype(f).copy()
    # deconv: shift s=(dy,dx); decw[s][c, (a*2+b)*3+o] = W[c,o,ky(a,u),kx(b,v)]
    dw = i['d0_dw']  # [128, 3, 4, 4]
    decw = np.zeros((9, 128, 12), f)
    for a in range(2):
        for u in range(2):
            ky = (1, 3)[u] if a == 0 else (0, 2)[u]
            dy = (0, -1)[u] if a == 0 else (1, 0)[u]
            for b in range(2):
                for v in range(2):
                    kx = (1, 3)[v] if b == 0 else (0, 2)[v]
                    dx = (0, -1)[v] if b == 0 else (1, 0)[v]
                    sidx = (dy + 1) * 3 + (dx + 1)
                    for o in range(3):
                        decw[sidx, :, (a * 2 + b) * 3 + o] += dw[:, o, ky, kx]
    # conv3 stacked-48: row ((a,dy),(b,dx),ch), col (class (a',b'), o)
    AY = [(0, 0), (0, 1), (1, -1), (1, 0)]
    cw = i['d0_cw']  # [o, ch, ky, kx]
    w3s = np.zeros((48, 12), h)
    for ap_ in range(2):
        for bp in range(2):
            for ky in range(3):
                va = ap_ + ky - 1
                a = va % 2
                dy = (va - a) // 2
                iy = AY.index((a, dy))
                for kx in range(3):
                    vb = bp + kx - 1
                    b = vb % 2
                    dx = (vb - b) // 2
                    jx = AY.index((b, dx))
                    row0 = 3 * (4 * iy + jx)
                    col0 = 6 * bp + 3 * ap_   # (b', a', o) column order
                    for o in range(3):
                        for ch in range(3):
                            w3s[row0 + ch, col0 + o] = h(cw[o, ch, ky, kx])
    bdec = np.zeros((12, 1), f)
    b3r = np.zeros((12, 1), f)
    for ab in range(4):
        bdec[3 * ab:3 * ab + 3, 0] = i['d0_db']
        b3r[3 * ab:3 * ab + 3, 0] = i['d0_cb']
    return dict(
        w1s=w1s, w1n=w1n, w2s=w2s, w2n3=w2n3, ckt=ckt, vmat=vmat,
        decw=np.ascontiguousarray(decw.transpose(1, 0, 2).reshape(128, 108)).astype(h),
        w3s=w3s,
        b1s=i['e0s_b1'].reshape(128, 1).astype(f),
        b1n=i['e0n_b1'].reshape(16, 1).astype(f),
        b2s=i['e0s_b2'].reshape(128, 1).astype(f),
        b2n=i['e0n_b2'].reshape(16, 1).astype(f),
        bdec=bdec, b3r=b3r,
    )


_last = {}


def last_exec_ns():
    return _last.get('ns')


def _get_runner():
    """Cached jitted SPMD callable over 8 cores (traced once)."""
    if 'runner' in _cache:
        return _cache['runner']
    import jax
    from jax.sharding import Mesh, PartitionSpec
    from jax.experimental.shard_map import shard_map
    from concourse import bass2jax, mybir as _mb
    nc = _cache['nc']
    bass2jax.install_neuronx_cc_hook()
    partition_name = nc.partition_id_tensor.name if nc.partition_id_tensor else None
    in_names, out_names, out_avals, zero_outs = [], [], [], []
    for alloc in nc.m.functions[0].allocations:
        if not isinstance(alloc, _mb.MemoryLocationSet):
            continue
        name = alloc.memorylocations[0].name
        if alloc.kind == "ExternalInput":
            if name != partition_name:
                in_names.append(name)
        elif alloc.kind == "ExternalOutput":
            shape = tuple(alloc.tensor_shape)
            dtype = _mb.dt.np(alloc.dtype)
            out_names.append(name)
            out_avals.append(jax.core.ShapedArray(shape, dtype))
            zero_outs.append(np.zeros(shape, dtype))
    n_params = len(in_names)
    n_outs = len(out_avals)
    all_names = list(in_names) + list(out_names)
    if partition_name is not None:
        all_names.append(partition_name)

    def _body(*args):
        operands = list(args)
        if partition_name is not None:
            operands.append(bass2jax.partition_id_tensor())
        outs = bass2jax._bass_exec_p.bind(
            *operands, out_avals=tuple(out_avals), in_names=tuple(all_names),
            out_names=tuple(out_names), lowering_input_output_aliases=(),
            sim_require_finite=True, sim_require_nnan=True, nc=nc)
        return tuple(outs)

    devices = jax.devices()[:N_CORES]
    mesh = Mesh(np.asarray(devices), ("core",))
    sharded = jax.jit(
        shard_map(_body, mesh=mesh,
                  in_specs=(PartitionSpec("core"),) * (n_params + n_outs),
                  out_specs=(PartitionSpec("core"),) * n_outs,
                  check_rep=False),
        keep_unused=True)

    from jax.sharding import NamedSharding
    sh = NamedSharding(mesh, PartitionSpec("core"))
    _cache['sharding'] = sh
    _cache['devices'] = devices
    _cache['runner'] = (sharded, in_names, out_names, out_avals, zero_outs)
    return _cache['runner']


def _make_global(per_core_arrs):
    """Assemble a sharded global array from per-core numpy shards (no
    on-device slicing)."""
    import jax
    sh = _cache['sharding']
    devices = _cache['devices']
    a0 = np.asarray(per_core_arrs[0])
    global_shape = (len(per_core_arrs) * a0.shape[0], *a0.shape[1:])
    bufs = [jax.device_put(np.ascontiguousarray(a), d)
            for a, d in zip(per_core_arrs, devices)]
    return jax.make_array_from_single_device_arrays(global_shape, sh, bufs)


def _run_fast(in_maps):
    import jax
    sharded, in_names, out_names, out_avals, zero_outs = _get_runner()
    if 'dev_zeros' not in _cache:
        _cache['dev_zeros'] = [
            _make_global([np.zeros(z.shape, z.dtype)] * N_CORES)
            for z in zero_outs]
    n_cores = len(in_maps)
    gin = [_make_global([in_maps[c][nm] for c in range(n_cores)])
           for nm in in_names]
    outs = sharded(*gin, *_cache['dev_zeros'])
    return [{nm: np.asarray(outs[i]).reshape(n_cores, *out_avals[i].shape)[c]
             for i, nm in enumerate(out_names)} for c in range(n_cores)]


def _build_tiny():
    nc = bacc.Bacc("TRN2", target_bir_lowering=False, name="tiny")
    xi = nc.dram_tensor("xi", [128, 128], f32, kind="ExternalInput")
    xo = nc.dram_tensor("xo", [128, 128], f32, kind="ExternalOutput")
    with tile.TileContext(nc) as tc:
        with tc.tile_pool(name="sb", bufs=1) as sb:
            t = sb.tile([128, 128], f32)
            nc.sync.dma_start(t[:], xi[:])
            nc.sync.dma_start(xo[:], t[:])
    nc.compile()
    return nc


def bench_hw(n_iter=12, **inputs):
    """Estimate device exec time: full-kernel min wall minus trivial-kernel
    min wall (same 8-core dispatch path)."""
    import time as _t, jax
    from jax.sharding import Mesh, PartitionSpec
    from jax.experimental.shard_map import shard_map
    from concourse import bass2jax
    if 'nc' not in _cache:
        _cache['nc'] = _build()
    shared = _prep_weights({k: np.asarray(v) for k, v in inputs.items()})
    x = np.asarray(inputs['x'], dtype=np.float32)
    in_maps = [dict(shared, x=np.ascontiguousarray(x[c])) for c in range(N_CORES)]
    sharded, in_names, out_names, out_avals, zero_outs = _get_runner()
    gin = [_make_global([in_maps[c][nm] for c in range(N_CORES)])
           for nm in in_names]
    gz = [_make_global([np.zeros(z.shape, z.dtype)] * N_CORES)
          for z in zero_outs]

    def mintime(fn, args):
        ts = []
        for _ in range(n_iter):
            t0 = _t.perf_counter()
            o = fn(*args)
            jax.block_until_ready(o)
            ts.append(_t.perf_counter() - t0)
        return min(ts), ts

    tfull, ts_full = mintime(sharded, (*gin, *gz))

    if 'tiny_fn' not in _cache:
        ncT = _build_tiny()
        bass2jax.install_neuronx_cc_hook()
        pn = ncT.partition_id_tensor.name if ncT.partition_id_tensor else None

        def _tb(xi, xoz):
            ops = [xi, xoz]
            if pn is not None:
                ops.append(bass2jax.partition_id_tensor())
            names = ["xi", "xo"] + ([pn] if pn else [])
            return tuple(bass2jax._bass_exec_p.bind(
                *ops,
                out_avals=(jax.core.ShapedArray((128, 128), np.float32),),
                in_names=tuple(names), out_names=("xo",),
                lowering_input_output_aliases=(),
                sim_require_finite=True, sim_require_nnan=True, nc=ncT))
        mesh = Mesh(np.asarray(_cache['devices']), ("core",))
        _cache['tiny_fn'] = jax.jit(shard_map(
            _tb, mesh=mesh, in_specs=(PartitionSpec("core"),) * 2,
            out_specs=(PartitionSpec("core"),), check_rep=False),
            keep_unused=True)
        _cache['tiny_in'] = (
            _make_global([np.zeros((128, 128), np.float32)] * N_CORES),
            _make_global([np.zeros((128, 128), np.float32)] * N_CORES))
    ttiny, ts_tiny = mintime(_cache['tiny_fn'], _cache['tiny_in'])
    return max(0.0, tfull - ttiny), tfull, ttiny


def bench(n_iter=20, **inputs):
    """Min wall time of the on-device executable (inputs pre-staged)."""
    import time as _t, jax
    if 'nc' not in _cache:
        _cache['nc'] = _build()
    shared = _prep_weights({k: np.asarray(v) for k, v in inputs.items()})
    x = np.asarray(inputs['x'], dtype=np.float32)
    in_maps = [dict(shared, x=np.ascontiguousarray(x[c])) for c in range(N_CORES)]
    sharded, in_names, out_names, out_avals, zero_outs = _get_runner()
    if 'dev_zeros' not in _cache:
        _cache['dev_zeros'] = [
            _make_global([np.zeros(z.shape, z.dtype)] * N_CORES)
            for z in zero_outs]
    gin = [_make_global([in_maps[c][nm] for c in range(N_CORES)])
           for nm in in_names]
    times = []
    for it in range(n_iter):
        t0 = _t.perf_counter()
        outs = sharded(*gin, *_cache['dev_zeros'])
        jax.block_until_ready(outs)
        times.append(_t.perf_counter() - t0)
    return min(times), times


def kernel(**inputs):
    if 'nc' not in _cache:
        _cache['nc'] = _build()
    nc = _cache['nc']
    shared = _prep_weights({k: np.asarray(v) for k, v in inputs.items()})
    x = np.asarray(inputs['x'], dtype=np.float32)
    in_maps = [dict(shared, x=np.ascontiguousarray(x[c])) for c in range(N_CORES)]
    res = _run_fast(in_maps)
    out = np.stack([res[c]["out"] for c in range(N_CORES)], axis=0)
    return out
